# revision 53
# baseline (speedup 1.0000x reference)
"""BiDAF-style attention kernel for Trainium2, data-parallel over batch on 8 cores.

Shapes (hardcoded): B=16, C=2048, Q=128, E=200, O=128.
Each core processes 2 batches. All matmuls in float32r (moving dim >= 256),
softmax without max-shift (scores ~ N(0,1), exp never overflows).

Layout: feature-on-partitions for everything contracted over e/f. Host ships
raw transposes of x_contexts / x_questions plus the natural layouts; the
projection computes out^T [o, c] and the host transposes back.

Softmax-factor cancellation: E/sum(E) is invariant to per-output-index
exponential offsets, so the q-softmax orientation folds only s_q (ACT exp
bias) and both orientations get s_c via lhsq = w3*xqT + w1; normalizers come
free (ones column in xcn for z, a ones-vector matmul for r, with 1/r
partition-broadcast as a K=1 matmul into PSUM).
"""

import numpy as np

import concourse.bass as bass
import concourse.mybir as mybir
from concourse import bacc
from concourse.bass import MemorySpace
from concourse.tile import TileContext
from concourse.bass_utils import run_bass_kernel_spmd

B, C, Q, E, O = 16, 2048, 128, 200, 128
NB = 2          # batches per core
NCORES = 8
EPAD = 256      # padded e/q for small-N matmuls (float32r needs N>=256)
CCH = 512       # c chunk size
NCH = C // CCH  # 4
CT = 128        # c tile (partitions)
NCT = C // CT   # 16
F32 = mybir.dt.float32
F32R = mybir.dt.float32r
EXP = mybir.ActivationFunctionType.Exp

_CACHE = {}


def _build(num_devices=NCORES, reps=1):
    nc = bacc.Bacc("TRN2", target_bir_lowering=False, debug=False,
                   num_devices=num_devices)

    d_xcta = nc.dram_tensor("xcta", [NB, 128, C], F32R, kind="ExternalInput")
    d_xctb = nc.dram_tensor("xctb", [NB, 73, C], F32R, kind="ExternalInput")
    # xcn_shuf[b, ct, p, :] = [x_contexts[b, ct*128+p, :], 1.0, 0-pad]
    d_xcn = nc.dram_tensor("xcn", [NB, NCT, 128, 204], F32R,
                           kind="ExternalInput")
    # xq_pack[b, p, :] = [wcols row p (6), xqT[0:128] row p,
    #                     xqT[128:200] row p (pad), xqn row p]
    d_xq = nc.dram_tensor("xq", [NB, 128, 462], F32R, kind="ExternalInput")
    # wp chunks padded to 128 rows each: [8, 128, O]
    d_wp = nc.dram_tensor("wp", [8, 128, O], F32R, kind="ExternalInput")
    d_out = nc.dram_tensor("out_t", [NB, O, C], F32, kind="ExternalOutput")

    # wp row chunks paired with featsT row chunks
    WP_CH = [(0, 128), (128, 201), (201, 329), (329, 401),
             (401, 529), (529, 601), (601, 729), (729, 801)]

    def mmr(ps, lhsT, rhs, start=True, stop=True):
        nc.tensor.matmul(ps, lhsT.bitcast(F32R), rhs.bitcast(F32R),
                         start=start, stop=stop)

    with TileContext(nc) as tc:
        with (
            tc.tile_pool(name="singles", bufs=1) as singles,
            tc.tile_pool(name="inputs", bufs=2) as inputs,
            tc.tile_pool(name="work", bufs=1) as work,
            tc.tile_pool(name="work2", bufs=2) as work2,
            tc.tile_pool(name="small", bufs=4) as small,
            tc.tile_pool(name="ps_big", bufs=3, space=MemorySpace.PSUM) as ps_big,
            tc.tile_pool(name="ps_t", bufs=1, space=MemorySpace.PSUM) as ps_t_pool,
            tc.tile_pool(name="ps_cq", bufs=2, space=MemorySpace.PSUM) as ps_cq,
            tc.tile_pool(name="ps_sc", bufs=2, space=MemorySpace.PSUM) as ps_sc,
                    ):
            # ---- batch-independent constants ----
            wp_all = singles.tile([128, 8, O], F32R, tag="wp")
            wp_sb = [wp_all[0:(r1 - r0), k, :]
                     for k, (r0, r1) in enumerate(WP_CH)]
            onecol = singles.tile([128, 1], F32R, tag="onecol")
            nc.vector.memset(onecol.bitcast(F32), 1.0)
            ones_row = singles.tile([1, 128], F32R, tag="ones_row")
            nc.vector.memset(ones_row.bitcast(F32), 1.0)

            for rep in range(reps):
              for b in range(NB):
                # ---- input DMAs (few + large; HWDGE costs 625ns each) ----
                xq = inputs.tile([128, 462], F32R, tag="xq")
                nc.sync.dma_start(out=xq, in_=d_xq.ap()[b])
                w1a = xq[:, 0:1]
                w1b = xq[0:72, 1:2]
                w2a = xq[:, 2:3]
                w2b = xq[0:72, 3:4]
                w3a = xq[:, 4:5]
                w3b = xq[0:72, 5:6]
                xqta = xq[:, 6:134]
                xqtb = xq[0:72, 134:262]
                xqn = xq[:, 262:462]
                xcta = inputs.tile([128, C], F32R, tag="xcta")
                xctb = inputs.tile([73, C], F32R, tag="xctb")
                for ch in range(NCH):
                    sl = slice(ch * CCH, (ch + 1) * CCH)
                    nc.sync.dma_start(out=xcta[:, sl],
                                      in_=d_xcta.ap()[b, :, sl])
                    nc.sync.dma_start(out=xctb[:, sl],
                                      in_=d_xctb.ap()[b, :, sl])
                xcn = inputs.tile([128, NCT, EPAD], F32R, tag="xcn")
                xcn_src = d_xcn.ap()[b].rearrange("n p m -> p n m")
                for g in range(4):
                    gs = slice(4 * g, 4 * g + 4)
                    nc.sync.dma_start(out=xcn[:, gs, 0:204],
                                      in_=xcn_src[:, gs, :])
                if rep == 0 and b == 0:
                    nc.sync.dma_start(out=wp_all,
                                      in_=d_wp.ap().rearrange("k p o -> p k o"))

                # ---- question-side lhsT prep (pure compute, no DMA) ----
                # lhsq[:, q] = w3*xqT[:, q] + w1  (the +w1 adds s_c to both
                # score orientations; it cancels in S1 = E/r where unwanted)
                lhsq_a = work.tile([128, EPAD], F32R, tag="lhsq_a")
                nc.vector.memset(lhsq_a.bitcast(F32), 0.0)
                nc.vector.scalar_tensor_tensor(
                    lhsq_a[:, 0:Q], xqta, w3a.bitcast(F32),
                    w1a.broadcast_to([128, Q]),
                    op0=mybir.AluOpType.mult, op1=mybir.AluOpType.add)
                lhsq_b = work.tile([72, EPAD], F32R, tag="lhsq_b")
                nc.vector.memset(lhsq_b.bitcast(F32), 0.0)
                nc.vector.scalar_tensor_tensor(
                    lhsq_b[:, 0:Q], xqtb, w3b.bitcast(F32),
                    w1b.broadcast_to([72, Q]),
                    op0=mybir.AluOpType.mult, op1=mybir.AluOpType.add)
                # s_q column [q, 1] = xqT^T @ w2 -> exp_qc bias.
                # (exp(s_c) factors cancel in S1 = E/r, exp(s_q) factors
                # cancel in S2 = E/z, so each orientation only needs its
                # per-contraction-index term.)
                ps_sqc = ps_sc.tile([Q, 4], F32, tag="ps_sc")
                mmr(ps_sqc[:, 0:2], xqta, xq[:, 2:4], start=True,
                    stop=False)
                mmr(ps_sqc[:, 0:2], xqtb, xq[0:72, 3:5], start=False,
                    stop=True)
                sq_col = small.tile([Q, 1], F32, tag="sq_col")
                nc.vector.tensor_copy(sq_col, ps_sqc[:, 0:1])

                # ---- scores^T [q, c]: exp + z accum; r row; S1^T, chunked --
                eqc = work2.tile([Q, C], F32R, tag="eqc")
                s1t = work2.tile([Q, C], F32R, tag="s1t")
                rrow = work.tile([1, C], F32R, tag="rrow")
                for ch in range(NCH):
                    sl = slice(ch * CCH, (ch + 1) * CCH)
                    ps = ps_big.tile([128, CCH], F32, tag="ps_big")
                    mmr(ps[0:Q, :], lhsq_a[:, 0:Q], xcta[:, sl],
                        start=True, stop=False)
                    mmr(ps[0:Q, :], lhsq_b[:, 0:Q], xctb[0:72, sl],
                        start=False, stop=True)
                    nc.scalar.activation(out=eqc[:, sl], in_=ps[0:Q, :],
                                         func=EXP, bias=sq_col)
                # ---- scores [c, q] per c-tile + exp -> E_cq; t accumulate --
                ecq = work.tile([128, NCT, Q], F32R, tag="ecq")
                for ct in range(NCT):
                    tsl = slice(ct * CT, (ct + 1) * CT)
                    pool = ps_cq if ct % 2 == 0 else ps_t_pool
                    tagn = "ps_cq" if ct % 2 == 0 else "ps_t"
                    ps = pool.tile([128, EPAD], F32, tag=tagn)
                    mmr(ps, xcta[:, tsl], lhsq_a, start=True, stop=False)
                    mmr(ps, xctb[0:72, tsl], lhsq_b, start=False, stop=True)
                    nc.scalar.activation(out=ecq[:, ct, :], in_=ps[:, 0:Q],
                                         func=EXP)
                # r chunk = colsum over q; 1/r; broadcast; S1T chunk.
                # Emitted after the cq-MMs: PE streams in program order and
                # the r-MMs depend on ACT exps; by now those are long done.
                for ch in range(NCH):
                    sl = slice(ch * CCH, (ch + 1) * CCH)
                    psr = ps_sc.tile([1, CCH], F32, tag="ps_sc")
                    mmr(psr, onecol, eqc[:, sl])
                    with nc.allow_low_precision(
                            reason="f32r==f32 bits; verifier type plumbing"):
                        nc.vector.reciprocal(rrow[:, sl], psr)
                for ch in range(NCH):
                    sl = slice(ch * CCH, (ch + 1) * CCH)
                    # broadcast 1/r across partitions via K=1 matmul
                    psb = ps_sc.tile([128, CCH], F32, tag="ps_sc")
                    mmr(psb, ones_row, rrow[:, sl])
                    nc.vector.tensor_mul(s1t[:, sl], eqc[:, sl], psb)
                ps_t = ps_t_pool.tile([Q, EPAD], F32, tag="ps_t")
                for ct in range(NCT):
                    mmr(ps_t[:, 0:EPAD], ecq[:, ct, :], xcn[:, ct, :],
                        start=(ct == 0), stop=(ct == NCT - 1))
                # z' comes free from the ones column (200) of xcn
                rz = small.tile([Q, 1], F32, tag="rz")
                nc.vector.reciprocal(rz, ps_t[:, E:E + 1])
                t_sb = work.tile([Q, E], F32R, tag="t_sb")
                nc.vector.tensor_scalar_mul(t_sb, ps_t[:, 0:E], rz)

                # ---- c2q^T [e, c] and products ----
                c2qt0 = work.tile([128, C], F32R, tag="c2qt0")
                c2qt1 = work.tile([72, C], F32R, tag="c2qt1")
                p10 = work.tile([128, C], F32R, tag="p10")
                p11 = work.tile([72, C], F32R, tag="p11")
                p20 = work.tile([128, C], F32R, tag="p20")
                p21 = work.tile([72, C], F32R, tag="p21")
                for ch in range(NCH):
                    sl = slice(ch * CCH, (ch + 1) * CCH)
                    for ec, (e0, e1) in enumerate([(0, 128), (128, 200)]):
                        ne = e1 - e0
                        c2qt = (c2qt0, c2qt1)[ec]
                        p1 = (p10, p11)[ec]
                        xct_sl = xcta[:, sl] if ec == 0 else xctb[0:72, sl]
                        ps = ps_big.tile([128, CCH], F32, tag="ps_big")
                        mmr(ps[0:ne, :], xqn[:, e0:e1], s1t[:, sl])
                        if ec == 0:
                            nc.vector.tensor_copy(c2qt[:, sl], ps[0:ne, :])
                        else:
                            nc.scalar.copy(c2qt[:, sl], ps[0:ne, :])
                        # product on gpsimd (both SBUF) to offload DVE
                        nc.gpsimd.tensor_mul(p1[:, sl], c2qt[:, sl], xct_sl)
                    for ec, (e0, e1) in enumerate([(0, 128), (128, 200)]):
                        ne = e1 - e0
                        p2 = (p20, p21)[ec]
                        xct_sl = xcta[:, sl] if ec == 0 else xctb[0:72, sl]
                        # borrow the cq pool's banks (idle in this phase)
                        ps = ps_cq.tile([128, CCH], F32, tag="ps_cq")
                        mmr(ps[0:ne, :], t_sb[:, e0:e1], s1t[:, sl])
                        nc.vector.tensor_mul(p2[:, sl], ps[0:ne, :], xct_sl)

                # ---- projection out^T [o, c] ----
                feat_chunks = [xcta, xctb, c2qt0, c2qt1, p10, p11, p20, p21]
                out_sb = work.tile([O, C], F32, tag="out_sb")
                for ch in range(NCH):
                    sl = slice(ch * CCH, (ch + 1) * CCH)
                    pool = ps_big if ch % 2 == 0 else ps_cq
                    tagn = "ps_big" if ch % 2 == 0 else "ps_cq"
                    ps = pool.tile([128, CCH], F32, tag=tagn)
                    for k in range(8):
                        mmr(ps[0:O, :], wp_sb[k], feat_chunks[k][:, sl],
                            start=(k == 0), stop=(k == 7))
                    nc.scalar.copy(out_sb[:, sl], ps[0:O, :])
                    nc.sync.dma_start(out=d_out.ap()[b][:, sl],
                                      in_=out_sb[:, sl])

    nc.compile()
    return nc


def _get_nc():
    if "nc" not in _CACHE:
        _CACHE["nc"] = _build()
    return _CACHE["nc"]


def kernel(x_contexts, x_questions, w_sim, w_proj, b_proj, _trace=False):
    x_contexts = np.ascontiguousarray(x_contexts, dtype=np.float32)
    x_questions = np.ascontiguousarray(x_questions, dtype=np.float32)
    w_sim = np.asarray(w_sim, dtype=np.float32)
    w_proj = np.asarray(w_proj, dtype=np.float32)
    b_proj = np.asarray(b_proj, dtype=np.float32)

    # host-side layout prep (no model math)
    xct = np.ascontiguousarray(x_contexts.transpose(0, 2, 1))  # [B, E, C]
    xqt = np.ascontiguousarray(x_questions.transpose(0, 2, 1))  # [B, E, Q]
    xctb = np.empty((B, 73, C), np.float32)
    xctb[:, 0:72] = xct[:, 128:200]
    xctb[:, 72] = 1.0          # ones row (pairs the bias row of wp)
    xcn = np.zeros((B, NCT, 128, 204), np.float32)
    xcn[:, :, :, 0:E] = x_contexts.reshape(B, NCT, 128, E)
    xcn[:, :, :, E] = 1.0  # ones column: t-matmul accumulates z' there
    xq_pack = np.zeros((B, 128, 462), np.float32)
    w1, w2, w3 = w_sim[0, 0:200], w_sim[0, 200:400], w_sim[0, 400:600]
    xq_pack[:, :, 0], xq_pack[:, 0:72, 1] = w1[0:128], w1[128:200]
    xq_pack[:, :, 2], xq_pack[:, 0:72, 3] = w2[0:128], w2[128:200]
    xq_pack[:, :, 4], xq_pack[:, 0:72, 5] = w3[0:128], w3[128:200]
    xq_pack[:, :, 6:134] = xqt[:, 0:128]
    xq_pack[:, 0:72, 134:262] = xqt[:, 128:200, :]
    xq_pack[:, :, 262:462] = x_questions
    wpfull = np.concatenate(
        [w_proj.T[0:200], b_proj[None, :], w_proj.T[200:800]], axis=0)
    WP_CH = [(0, 128), (128, 201), (201, 329), (329, 401),
             (401, 529), (529, 601), (601, 729), (729, 801)]
    wp = np.zeros((8, 128, O), np.float32)
    for k, (r0, r1) in enumerate(WP_CH):
        wp[k, 0:r1 - r0] = wpfull[r0:r1]

    in_maps = []
    for c in range(NCORES):
        bs = slice(c * NB, (c + 1) * NB)
        in_maps.append({
            "xcta": np.ascontiguousarray(xct[bs, 0:128]),
            "xctb": np.ascontiguousarray(xctb[bs]),
            "xcn": np.ascontiguousarray(xcn[bs]),
            "xq": np.ascontiguousarray(xq_pack[bs]),
            "wp": wp,
        })

    nc = _get_nc()
    res = run_bass_kernel_spmd(nc, in_maps, core_ids=list(range(NCORES)),
                               trace=_trace)
    _CACHE["last_res"] = res

    out = np.empty((B, C, O), np.float32)
    for c in range(NCORES):
        ot = res.results[c]["out_t"]  # [NB, O, C]
        for b in range(NB):
            out[c * NB + b] = ot[b].T
    return out
